# revision 54
# baseline (speedup 1.0000x reference)
"""BiMamba Trainium2 kernel (v2, pipelined).

Sharding: 8 cores = (batch 2) x (direction 2) x (head-half 2). Each core runs an
identical SPMD Bass program on its slice: x[b]^T (time-flipped for bwd), in_proj
rows for its 12 heads (+ shared B/C rows). Per-core output: unnormalized
projected partial (2048, 768) + partial sum-of-squares; the RMSNorm rsqrt
commutes with the linear projection, so the host applies it to summed partials.

Scan: chunked SSD, chunk=128, with dt folded multiplicatively into x:
  y_t = sz_t [ exp(Acum_t) (C_t.h_prev + sum_{s<=t} (B_s.C_t) e^{-Acum_s} dt_s x_s)
               + Dp x_t ]
Decay plane D[s,t] = Acum_t - Acum_s via one K=39 bf16 matmul per chunk
(3 ones rows x 3-way bf16 split of +Acum_t, 36 rows of the same splits against
a constant -1 per-head indicator); exp(min(D,25)) on ACT, causal CB mask on DVE.

Schedule: one-block-delayed software pipeline - per 256-step block t the
emission is [dt-proj+softplus(t) | D-plane+state pre(t-1) x2 | chunk mains(t-1)
x2 | staging(t) | z/conv/transpose(t)] so PE's dense projection work overlaps
the DVE/ACT-bound scan chains, PE stays in high p-state, and the ACT stream
groups into [exp][ln][silu] blocks (3 act-table loads per block). All per-chunk
matmul operands in bf16; C.h_prev accumulated into the intra-chunk y PSUM;
broadcasts via k=1/ones matmuls; D-operands staged via 5 HWDGE DMAs per block;
weights loaded in single strided DMAs.

TimelineSim: 295.9 us per core (baseline 605.5 us, 2.05x); HW rel err 6.7e-3.
"""
import numpy as np
from contextlib import ExitStack

import concourse.bass as bass
import concourse.tile as tile
from concourse import bacc, mybir
from concourse.bass_utils import run_bass_kernel_spmd
from concourse.masks import make_identity

FP32 = mybir.dt.float32
FP32R = mybir.dt.float32r
BF16 = mybir.dt.bfloat16
AF = mybir.ActivationFunctionType
ALU = mybir.AluOpType

D_MODEL = 768
D_STATE = 16
HEADDIM = 64
D_CONV = 4
SEQ = 2048
NH = 12                  # heads per core
HH = NH * HEADDIM        # 768 x-channels per core
CMJ = HH + 2 * D_STATE   # 800 c-major feats: [x 768 | B 16 | C 16]
TMJ = HH                 # 768 t-major feats: z only
CH = 128
NCHUNK = SEQ // CH       # 16
TB = 256                 # time block
NTB = SEQ // TB
CPB = TB // CH           # 2
NKT = 6                  # d_model k-tiles
EPS = 1e-5
P = 128


def _rep(ap_tile, outer_count, inner_count, outer_step, inner_step, col0=0):
    """free-pattern AP helper on a 2D tile: [[pstep,P],[outer],[inner]]"""
    return bass.AP(tensor=ap_tile.tensor, offset=ap_tile.offset + col0,
                   ap=[[ap_tile.ap[0][0], ap_tile.ap[0][1]],
                       [outer_step, outer_count], [inner_step, inner_count]])


def build_program():
    nc = bacc.Bacc("TRN2", target_bir_lowering=False, debug=False, num_devices=8)

    def din(name, shape, dt=FP32):
        return nc.dram_tensor(name, shape, dt, kind="ExternalInput").ap()

    d_xT = din("xT", (D_MODEL, SEQ), FP32R)
    d_Wc = din("Wc", (D_MODEL, CMJ), FP32R)
    d_Wt = din("Wt", (D_MODEL, TMJ), FP32R)
    d_Wdt = din("Wdt", (D_MODEL, NH), FP32R)
    d_DIAGW = din("DIAGW", (D_CONV, NKT, P, P), FP32R)     # x-part diag tiles
    d_DIAGB = din("DIAGB", (D_CONV, P, D_STATE), FP32R)    # B: in-rows 0..15 -> out 0..15
    d_DIAGC = din("DIAGC", (D_CONV, P, D_STATE), FP32R)    # C: in-rows 16..31 -> out 0..15
    d_CONVBX = din("CONVBX", (P, NKT))                      # x-part conv bias per c-tile
    d_CONVBB = din("CONVBB", (D_STATE, 1))
    d_CONVBC = din("CONVBC", (D_STATE, 1))
    d_DTB_BC = din("DTB_BC", (P, NH))
    d_ANEG_BC = din("ANEG_BC", (P, NH))
    d_TRI = din("TRI", (P, P))                              # tri[s,t]=1 if s<=t
    d_ONES3 = din("ONES3", (3, TB), BF16)
    d_RHSC = din("RHSC", (3 * NH, NH * TB), BF16)           # row h*3+j -> head h block
    d_DPBIG = din("DPBIG", (P, HH), BF16)
    d_WCOMB = din("WCOMB", (HH, D_MODEL), BF16)
    d_OUT1 = nc.dram_tensor("OUT1", (SEQ, D_MODEL), FP32, kind="ExternalOutput").ap()
    d_OUT2 = nc.dram_tensor("OUT2", (P, NCHUNK), FP32, kind="ExternalOutput").ap()

    with tile.TileContext(nc, trace_sim=False) as tc, ExitStack() as ctx:
        const = ctx.enter_context(tc.tile_pool(name="const", bufs=1))
        wgt = ctx.enter_context(tc.tile_pool(name="wgt", bufs=1))
        seqp = ctx.enter_context(tc.tile_pool(name="seqp", bufs=1))
        spl1 = ctx.enter_context(tc.tile_pool(name="spl1", bufs=1))
        stg = ctx.enter_context(tc.tile_pool(name="stg", bufs=2))
        blk1 = ctx.enter_context(tc.tile_pool(name="blk1", bufs=2))
        blk2 = ctx.enter_context(tc.tile_pool(name="blk2", bufs=2))
        chk = ctx.enter_context(tc.tile_pool(name="chk", bufs=2))
        st = ctx.enter_context(tc.tile_pool(name="st", bufs=2))
        psA = ctx.enter_context(tc.tile_pool(name="psA", bufs=4, space="PSUM"))
        psY = ctx.enter_context(tc.tile_pool(name="psY", bufs=1, space="PSUM"))
        psW = ctx.enter_context(tc.tile_pool(name="psW", bufs=1, space="PSUM"))

        # ---- constants ----
        tri = const.tile([P, P], FP32); nc.sync.dma_start(tri[:], d_TRI)
        dpbig = const.tile([P, HH], BF16); nc.sync.dma_start(dpbig[:], d_DPBIG)
        convbx = const.tile([P, NKT], FP32); nc.sync.dma_start(convbx[:], d_CONVBX)
        convbb = const.tile([D_STATE, 1], FP32); nc.sync.dma_start(convbb[:], d_CONVBB)
        convbc = const.tile([D_STATE, 1], FP32); nc.sync.dma_start(convbc[:], d_CONVBC)
        dtb_bc = const.tile([P, NH], FP32); nc.sync.dma_start(dtb_bc[:], d_DTB_BC)
        aneg_bc = const.tile([P, NH], FP32); nc.sync.dma_start(aneg_bc[:], d_ANEG_BC)
        idn = const.tile([P, P], FP32); make_identity(nc, idn)
        idnr = const.tile([P, P], FP32R); nc.vector.tensor_copy(idnr[:], idn[:])
        idnb = const.tile([P, P], BF16); nc.vector.tensor_copy(idnb[:], idn[:])
        ones1f = const.tile([1, P], FP32); nc.vector.memset(ones1f[:], 1.0)
        ones1 = const.tile([1, P], FP32R); nc.vector.tensor_copy(ones1[:], ones1f[:])
        onesp = const.tile([P, P], FP32); nc.vector.memset(onesp[:], 1.0)

        wdtall = wgt.tile([P, NKT * NH], FP32R, tag="wdtall")
        nc.sync.dma_start(
            bass.AP(tensor=wdtall.tensor, offset=wdtall.offset,
                    ap=[[wdtall.ap[0][0], P], [NH, NKT], [1, NH]]),
            bass.AP(tensor=d_Wdt.tensor, offset=0,
                    ap=[[NH, P], [NH * P, NKT], [1, NH]]))
        wdt = [wdtall[:, kt * NH:(kt + 1) * NH] for kt in range(NKT)]
        wtall = wgt.tile([P, NKT * TMJ], FP32R, tag="wtall")
        nc.sync.dma_start(
            bass.AP(tensor=wtall.tensor, offset=wtall.offset,
                    ap=[[wtall.ap[0][0], P], [TMJ, NKT], [1, TMJ]]),
            bass.AP(tensor=d_Wt.tensor, offset=0,
                    ap=[[TMJ, P], [TMJ * P, NKT], [1, TMJ]]))
        wt = [wtall[:, kt * TMJ:(kt + 1) * TMJ] for kt in range(NKT)]
        wcall = wgt.tile([P, NKT * CMJ], FP32R, tag="wcall")
        nc.sync.dma_start(
            bass.AP(tensor=wcall.tensor, offset=wcall.offset,
                    ap=[[wcall.ap[0][0], P], [CMJ, NKT], [1, CMJ]]),
            bass.AP(tensor=d_Wc.tensor, offset=0,
                    ap=[[CMJ, P], [CMJ * P, NKT], [1, CMJ]]))
        wc = [wcall[:, kt * CMJ:(kt + 1) * CMJ] for kt in range(NKT)]
        dwall = wgt.tile([P, D_CONV * NKT * P], FP32R, tag="dwall")
        nc.sync.dma_start(
            bass.AP(tensor=dwall.tensor, offset=dwall.offset,
                    ap=[[dwall.ap[0][0], P], [P, D_CONV * NKT], [1, P]]),
            bass.AP(tensor=d_DIAGW.tensor, offset=0,
                    ap=[[P, P], [P * P, D_CONV * NKT], [1, P]]))
        diagw = [[dwall[:, (k * NKT + ct) * P:(k * NKT + ct + 1) * P]
                  for ct in range(NKT)] for k in range(D_CONV)]
        dball = wgt.tile([P, D_CONV * D_STATE], FP32R, tag="dball")
        nc.sync.dma_start(
            bass.AP(tensor=dball.tensor, offset=dball.offset,
                    ap=[[dball.ap[0][0], P], [D_STATE, D_CONV], [1, D_STATE]]),
            bass.AP(tensor=d_DIAGB.tensor, offset=0,
                    ap=[[D_STATE, P], [D_STATE * P, D_CONV], [1, D_STATE]]))
        diagb = [dball[:, k * D_STATE:(k + 1) * D_STATE] for k in range(D_CONV)]
        dcall = wgt.tile([P, D_CONV * D_STATE], FP32R, tag="dcall")
        nc.sync.dma_start(
            bass.AP(tensor=dcall.tensor, offset=dcall.offset,
                    ap=[[dcall.ap[0][0], P], [D_STATE, D_CONV], [1, D_STATE]]),
            bass.AP(tensor=d_DIAGC.tensor, offset=0,
                    ap=[[D_STATE, P], [D_STATE * P, D_CONV], [1, D_STATE]]))
        diagc = [dcall[:, k * D_STATE:(k + 1) * D_STATE] for k in range(D_CONV)]
        woall = wgt.tile([P, NKT * D_MODEL], BF16, tag="woall")
        nc.sync.dma_start(
            bass.AP(tensor=woall.tensor, offset=woall.offset,
                    ap=[[woall.ap[0][0], P], [D_MODEL, NKT], [1, D_MODEL]]),
            bass.AP(tensor=d_WCOMB.tensor, offset=0,
                    ap=[[D_MODEL, P], [D_MODEL * P, NKT], [1, D_MODEL]]))
        wcomb = [woall[:, ct * D_MODEL:(ct + 1) * D_MODEL] for ct in range(NKT)]

        ssqall = seqp.tile([P, NCHUNK], FP32)
        hN = None
        xbc = None

        def load_xtb(t0):
            tiles = []
            for kt in range(NKT):
                x = blk1.tile([P, TB], FP32R, tag=f"xtb{kt}")
                nc.sync.dma_start(x[:], d_xT[kt * P:(kt + 1) * P, t0:t0 + TB])
                tiles.append(x)
            return tiles

        def A_dt(tb, xtb):
            t0 = tb * TB
            st_d = {"t0": t0, "xtb": xtb}

            # ---- dt projection (tiny, overlaps prev block's chunks) ----
            dtall = blk1.tile([P, CPB * NH], FP32, tag="dtall")
            pdt = psA.tile([P, 512], FP32, tag="psA")
            for tt in range(CPB):
                for kt in range(NKT):
                    nc.tensor.matmul(pdt[:, tt * NH:(tt + 1) * NH],
                                     xtb[kt][:, tt * P:(tt + 1) * P], wdt[kt][:],
                                     start=(kt == 0), stop=(kt == NKT - 1))
            nc.vector.tensor_copy(dtall[:], pdt[:, 0:CPB * NH])

            # ---- dt path (part 1) ----
            tmp = chk.tile([P, CPB * NH], FP32, tag="dtt")
            nc.vector.tensor_tensor(tmp[:], dtall[:], _rep(dtb_bc, CPB, NH, 0, 1), ALU.add)
            spe = chk.tile([P, CPB * NH], FP32, tag="spe")
            nc.scalar.activation(spe[:], tmp[:], AF.Exp)
            st_d["spe"] = spe
            return st_d

        def A_dt2(st_d):
            spe = st_d["spe"]
            sp = chk.tile([P, CPB * NH], FP32, tag="sp")
            nc.scalar.activation(sp[:], spe[:], AF.Ln, bias=1.0)
            logda = chk.tile([P, CPB * NH], FP32, tag="logda")
            nc.vector.tensor_tensor(logda[:], sp[:], _rep(aneg_bc, CPB, NH, 0, 1), ALU.mult)
            acum = chk.tile([P, CPB * NH], FP32R, tag="acum")
            ldac = spl1.tile([NH, TB], FP32, tag="ldac")
            for i in range(CPB):
                acs = slice(i * NH, (i + 1) * NH)
                pa = psA.tile([P, 512], FP32, tag="psA")
                nc.tensor.matmul(pa[:, 0:NH], tri[:], logda[:, acs], start=True, stop=True)
                nc.tensor.matmul(pa[0:NH, 128:128 + P], logda[:, acs], tri[:],
                                 start=True, stop=True)
                nc.vector.tensor_copy(acum[:, acs], pa[:, 0:NH])
                nc.vector.tensor_copy(ldac[:, i * P:(i + 1) * P], pa[0:NH, 128:128 + P])
            spb = chk.tile([P, CPB * NH], BF16, tag="spb")
            nc.vector.tensor_copy(spb[:], sp[:])
            # 3-way bf16 split of Acum (c-major); ld rows = -ac via -1 indicator
            spl = spl1.tile([NH, 3 * TB], BF16, tag="spl")
            r1 = spl1.tile([NH, TB], FP32, tag="r1")
            nc.gpsimd.tensor_copy(spl[:, 0:TB], ldac[:])
            nc.gpsimd.tensor_tensor(r1[:], ldac[:], spl[:, 0:TB], ALU.subtract)
            nc.gpsimd.tensor_copy(spl[:, TB:2 * TB], r1[:])
            nc.gpsimd.tensor_tensor(r1[:], r1[:], spl[:, TB:2 * TB], ALU.subtract)
            nc.gpsimd.tensor_copy(spl[:, 2 * TB:3 * TB], r1[:])
            st_d.update(acum=acum, logda=logda, spb=spb)

            # ---- stage D-matmul operands (HWDGE) ----
            lhsD = stg.tile([3 + 3 * NH, TB], BF16, tag="lhsD")
            nc.gpsimd.memset(lhsD[0:3, :], 1.0)
            nc.sync.dma_start(
                lhsD[3:3 + 3 * NH, :],
                bass.AP(tensor=spl.tensor, offset=spl.offset,
                        ap=[[spl.ap[0][0], NH], [TB, 3], [1, TB]]))
            rhsD = rhsD_bufs[tb % 2]
            for j in range(3):
                nc.sync.dma_start(
                    bass.AP(tensor=rhsD.tensor, offset=rhsD.offset + j * rhsD.ap[0][0],
                            ap=[[rhsD.ap[0][0], 1], [TB, NH], [1, TB]]),
                    bass.AP(tensor=spl.tensor,
                            offset=spl.offset + j * TB,
                            ap=[[spl.ap[0][0], NH], [1, TB]]))
            st_d.update(lhsD=lhsD, rhsD=rhsD)

            return st_d

        def A_z(st_d):
            xtb = st_d["xtb"]
            # ---- in_proj t-major z; silu straight out of PSUM ----
            sztiles = []
            for tt in range(CPB):
                sz = blk1.tile([P, HH], BF16, tag=f"sz{tt}")
                for nb in range(2):
                    f0 = nb * 384
                    p = psA.tile([P, 512], FP32, tag="psA")
                    for kt in range(NKT):
                        nc.tensor.matmul(p[:, 0:384], xtb[kt][:, tt * P:(tt + 1) * P],
                                         wt[kt][:, f0:f0 + 384],
                                         start=(kt == 0), stop=(kt == NKT - 1))
                    nc.scalar.activation(sz[:, f0:f0 + 384], p[:, 0:384], AF.Silu)
                sztiles.append(sz)

            st_d["sztiles"] = sztiles

        def A_cmaj(st_d):
            xtb = st_d["xtb"]
            # ---- in_proj c-major (conv input tiles, left-pad 3) ----
            nonlocal xbc
            xbc_prev = xbc
            xbc_new = []
            for ct in range(NKT + 1):
                cw = P if ct < NKT else CMJ - NKT * P   # 32 in last tile
                p = psA.tile([P, 512], FP32, tag="psA")
                for kt in range(NKT):
                    nc.tensor.matmul(p[:cw, 0:TB], wc[kt][:, ct * P:ct * P + cw],
                                     xtb[kt][:], start=(kt == 0), stop=(kt == NKT - 1))
                xb = blk2.tile([P, TB + 3], FP32R, tag=f"xbc{ct}")
                if xbc_prev is None:
                    nc.vector.memset(xb[:cw, 0:3].bitcast(FP32), 0.0)
                else:
                    nc.vector.tensor_copy(xb[:cw, 0:3], xbc_prev[ct][:cw, TB:TB + 3])
                if ct % 2 == 0:
                    nc.vector.tensor_copy(xb[:cw, 3:], p[:cw, 0:TB])
                else:
                    nc.scalar.copy(xb[:cw, 3:], p[:cw, 0:TB])
                xbc_new.append(xb)
            xbc = xbc_new

            st_d["xbc"] = xbc_new

        def A_conv(st_d):
            xbc = st_d["xbc"]
            # ---- conv (diag matmuls) + silu ----
            xsil = []
            for ct in range(NKT):
                p = psA.tile([P, 512], FP32, tag="psA")
                for k in range(D_CONV):
                    nc.tensor.matmul(p[:, 0:TB], diagw[k][ct][:], xbc[ct][:, k:k + TB],
                                     start=(k == 0), stop=(k == D_CONV - 1))
                xsl = blk1.tile([P, TB], FP32R, tag=f"xsil{ct}")
                nc.scalar.activation(xsl[:], p[:, 0:TB], AF.Silu,
                                     bias=convbx[:, ct:ct + 1], scale=1.0)
                xsil.append(xsl)
            bsil = blk1.tile([D_STATE, TB], FP32R, tag="bsil")
            csil = blk1.tile([D_STATE, TB], FP32R, tag="csil")
            for dst, dg, bias in ((bsil, diagb, convbb), (csil, diagc, convbc)):
                p = psA.tile([P, 512], FP32, tag="psA")
                for k in range(D_CONV):
                    nc.tensor.matmul(p[:D_STATE, 0:TB], dg[k][0:32, :], xbc[NKT][0:32, k:k + TB],
                                     start=(k == 0), stop=(k == D_CONV - 1))
                nc.scalar.activation(dst[:], p[:D_STATE, 0:TB], AF.Silu,
                                     bias=bias[:], scale=1.0)

            st_d["xsil"] = xsil
            st_d["bsil"] = bsil
            st_d["csil"] = csil

        def A_tr(st_d):
            xsil = st_d["xsil"]
            bsil = st_d["bsil"]
            sztiles = st_d["sztiles"]
            acum = st_d["acum"]
            spb = st_d["spb"]
            expac = chk.tile([P, CPB * NH], BF16, tag="expac")
            nc.scalar.activation(expac[:], acum[:], AF.Exp)

            # ---- transpose x + B to s-major (bf16) + per-chunk gate tiles ----
            xs_tiles = []
            e4pre_tiles = []
            sze_tiles = []
            xdt_tiles = []
            for tt in range(CPB):
                xst = blk2.tile([P, HH + D_STATE], BF16, tag=f"xst{tt}")
                for g in range(2):  # two groups of 3 transposes + (B on 2nd)
                    pt = psA.tile([P, 512], FP32, tag="psA")
                    for k in range(3):
                        ct = g * 3 + k
                        nc.tensor.transpose(pt[:, k * P:(k + 1) * P].bitcast(FP32R),
                                            xsil[ct][:, tt * P:(tt + 1) * P], idnr[:])
                    if g == 1:
                        nc.tensor.transpose(pt[:, 3 * P:3 * P + D_STATE].bitcast(FP32R),
                                            bsil[:, tt * P:(tt + 1) * P],
                                            idnr[0:D_STATE, 0:D_STATE])
                        nc.scalar.copy(xst[:, g * 384:g * 384 + 384 + D_STATE],
                                       pt[:, 0:384 + D_STATE])
                    else:
                        nc.scalar.copy(xst[:, 0:384], pt[:, 0:384])
                xs_tiles.append(xst)
                e4p = chk.tile([P, HH], BF16, tag=f"e4p{tt}")
                nc.gpsimd.tensor_tensor(e4p[:], xst[:, 0:HH], dpbig[:], ALU.mult)
                nc.gpsimd.tensor_tensor(e4p[:], e4p[:], sztiles[tt][:], ALU.mult)
                e4pre_tiles.append(e4p)
                sze = chk.tile([P, HH], BF16, tag=f"sze{tt}")
                nc.vector.tensor_tensor(
                    sze[:], sztiles[tt][:],
                    bass.AP(tensor=expac.tensor, offset=expac.offset + tt * NH,
                            ap=[[expac.ap[0][0], P], [1, NH], [0, HEADDIM]]),
                    ALU.mult)
                sze_tiles.append(sze)
                xdt = chk.tile([P, HH], BF16, tag=f"xdt{tt}")
                nc.vector.tensor_tensor(
                    xdt[:], xst[:, 0:HH],
                    bass.AP(tensor=spb.tensor, offset=spb.offset + tt * NH,
                            ap=[[spb.ap[0][0], P], [1, NH], [0, HEADDIM]]),
                    ALU.mult)
                xdt_tiles.append(xdt)

            st_d.update(xs=xs_tiles, e4p=e4pre_tiles, sze=sze_tiles, xdt=xdt_tiles)

        def C_pre(sd, i):
            t0 = sd["t0"]
            lhsD, rhsD = sd["lhsD"], sd["rhsD"]
            acum, logda = sd["acum"], sd["logda"]
            bsil, csil = sd["bsil"], sd["csil"]
            xst = sd["xs"][i]
            if True:
                ci = (t0 // P) + i
                xst = sd["xs"][i]
                acs = slice(i * NH, (i + 1) * NH)

                # C.B^T causal mask (bf16)
                pcbt = psA.tile([P, 512], FP32, tag="psA")
                nc.tensor.matmul(pcbt[:, 0:P], bsil[:, i * P:(i + 1) * P],
                                 csil[:, i * P:(i + 1) * P], start=True, stop=True)
                cbtm = chk.tile([P, P], BF16, tag="cbtm")
                nc.vector.tensor_tensor(cbtm[:], pcbt[:, 0:P], tri[:], ALU.mult)

                # D matmuls: K=39 bf16; exp then causal CB mask in place
                lall = chk.tile([P, NH * CH], BF16, tag="lall")
                for nb in range(3):
                    pd = psA.tile([P, 512], FP32, tag="psA")
                    nc.tensor.matmul(
                        pd[:],
                        lhsD[:, i * P:(i + 1) * P],
                        bass.AP(tensor=rhsD.tensor,
                                offset=rhsD.offset + nb * 4 * TB + i * P,
                                ap=[[rhsD.ap[0][0], 3 + 3 * NH], [TB, 4], [1, P]]),
                        start=True, stop=True)
                    sl = lall[:, nb * 512:(nb + 1) * 512]
                    nc.vector.tensor_scalar_min(sl, pd[:], 25.0)
                    nc.scalar.activation(sl, sl, AF.Exp)
                    nc.vector.tensor_tensor(sl, _rep(cbtm, 4, CH, 0, 1, col0=0),
                                            sl, ALU.mult)

                # ws = exp(Aend - Acum); Aend = colsum(logda) bcast via ones matmul
                pb = psA.tile([P, 512], FP32, tag="psA")
                nc.tensor.matmul(pb[:, 0:NH], onesp[:], logda[:, acs],
                                 start=True, stop=True)
                ws = chk.tile([P, NH], FP32, tag="ws")
                nc.vector.tensor_tensor(ws[:], pb[:, 0:NH], acum[:, acs], ALU.subtract)
                nc.scalar.activation(ws[:], ws[:], AF.Exp)
                eae = chk.tile([1, NH], FP32R, tag="eae")
                nc.scalar.activation(eae[:], pb[0:1, 0:NH], AF.Exp)
                pe2 = psA.tile([P, 512], FP32, tag="psA")
                nc.tensor.matmul(pe2[0:D_STATE, 0:NH], ones1[0:1, 0:D_STATE],
                                 eae[:], start=True, stop=True)
                eaebc = chk.tile([D_STATE, NH], FP32, tag="eaebc")
                nc.scalar.copy(eaebc[:], pe2[0:D_STATE, 0:NH])
                sd["eaebc%d" % i] = eaebc
                bd = chk.tile([P, NH * D_STATE], BF16, tag="bd")
                nc.vector.tensor_tensor(
                    bd[:],
                    bass.AP(tensor=xst.tensor, offset=xst.offset + HH,
                            ap=[[xst.ap[0][0], P], [0, NH], [1, D_STATE]]),
                    bass.AP(tensor=ws.tensor, offset=ws.offset,
                            ap=[[ws.ap[0][0], P], [1, NH], [0, D_STATE]]),
                    ALU.mult)

            sd["cbtm%d" % i] = cbtm
            sd["lall%d" % i] = lall
            sd["ws%d" % i] = ws
            sd["bd%d" % i] = bd
            sd["pb%d" % i] = pb

        def C_main(sd, i):
            nonlocal hN
            t0 = sd["t0"]
            acum, logda = sd["acum"], sd["logda"]
            bsil, csil = sd["bsil"], sd["csil"]
            xst = sd["xs"][i]
            ci = (t0 // P) + i
            acs = slice(i * NH, (i + 1) * NH)
            lall = sd["lall%d" % i]
            bd = sd["bd%d" % i]
            pb = sd["pb%d" % i]
            if True:
                # y = (C.h_prev + sum_s M dt x) all accumulated in one PSUM
                hN_prev = hN
                py = psY.tile([P, HH], FP32, tag="psY")
                if hN_prev is not None:
                    nc.tensor.matmul(py[:, 0:512], csil[:, i * P:(i + 1) * P],
                                     hN_prev[:, 0:512], start=True, stop=False)
                    nc.tensor.matmul(py[:, 512:HH], csil[:, i * P:(i + 1) * P],
                                     hN_prev[:, 512:HH], start=True, stop=False)
                xdt = sd["xdt"][i]
                for h in range(NH):
                    nc.tensor.matmul(py[:, h * 64:(h + 1) * 64],
                                     lall[:, h * CH:(h + 1) * CH],
                                     xdt[:, h * 64:(h + 1) * 64],
                                     start=(hN_prev is None), stop=True)
                pst = psW.tile([P, HH], FP32, tag="pwst")
                for h in range(NH):
                    nc.tensor.matmul(pst[0:D_STATE, h * 64:(h + 1) * 64],
                                     bd[:, h * D_STATE:(h + 1) * D_STATE],
                                     xdt[:, h * 64:(h + 1) * 64], start=True, stop=True)
                # epilogue first (keeps DVE queue clear of the state chain):
                # yg = py * (sz exp(Acum)) + (Dp x) sz
                yg1 = chk.tile([P, HH], BF16, tag="e1")
                nc.vector.tensor_tensor(yg1[:], py[:], sd["sze"][i][:], ALU.mult)
                yg = chk.tile([P, HH], BF16, tag="yg")
                nc.vector.tensor_tensor(yg[:], yg1[:], sd["e4p"][i][:], ALU.add)

                # state carry: hN = hN_prev * exp(Aend) + pst
                hN_new = st.tile([D_STATE, HH], FP32R, tag="hN")
                if hN_prev is None:
                    nc.vector.tensor_copy(hN_new[:], pst[0:D_STATE, :])
                else:
                    eaebc = sd["eaebc%d" % i]
                    nc.gpsimd.tensor_tensor(
                        hN_new[:], hN_prev[:],
                        bass.AP(tensor=eaebc.tensor, offset=eaebc.offset,
                                ap=[[eaebc.ap[0][0], D_STATE], [1, NH], [0, HEADDIM]]),
                        ALU.mult)
                    nc.vector.tensor_tensor(hN_new[:], hN_new[:], pst[0:D_STATE, :], ALU.add)
                hN = hN_new

                # out projection: transpose yg, accumulate W^T y
                pw = psW.tile([P, D_MODEL], FP32, tag="pwst")
                ygts = []
                for g in range(2):
                    ptr = psA.tile([P, 512], FP32, tag="psA")
                    for k in range(3):
                        ct = g * 3 + k
                        nc.tensor.transpose(ptr[:, k * 64:(k + 1) * 64].bitcast(BF16),
                                            yg[:, ct * P:(ct + 1) * P], idnb[:])
                    ygt = chk.tile([P, 384], BF16, tag=f"ygt{g}")
                    if g == 0:
                        nc.scalar.copy(ygt[:], ptr[:, 0:192].bitcast(BF16))
                    else:
                        nc.vector.tensor_copy(ygt[:], ptr[:, 0:192].bitcast(BF16))
                    ygts.append(ygt)
                for ct in range(NKT):
                    ygt_sl = ygts[ct // 3][:, (ct % 3) * P:(ct % 3 + 1) * P]
                    nc.tensor.matmul(pw[:, 0:512], ygt_sl, wcomb[ct][:, 0:512],
                                     start=(ct == 0), stop=(ct == NKT - 1))
                    nc.tensor.matmul(pw[:, 512:D_MODEL], ygt_sl, wcomb[ct][:, 512:D_MODEL],
                                     start=(ct == 0), stop=(ct == NKT - 1))
                o1 = chk.tile([P, D_MODEL], FP32, tag="o1")
                nc.scalar.copy(o1[:, 0:384], pw[:, 0:384])
                nc.vector.tensor_copy(o1[:, 384:768], pw[:, 384:768])
                nc.scalar.dma_start(d_OUT1[ci * P:(ci + 1) * P, :], o1[:])
                sqs = chk.tile([P, HH], BF16, tag="yg")
                nc.scalar.activation(sqs[:], yg[:], AF.Square,
                                     accum_out=ssqall[:, ci:ci + 1])


        # two explicit rhsD buffers; const indicator rows loaded once into each
        rhsD_bufs = []
        for rb in range(2):
            r0 = seqp.tile([3 + 3 * NH, NH * TB], BF16, tag=f"rhsD{rb}")
            nc.sync.dma_start(r0[3:, :], d_RHSC)
            rhsD_bufs.append(r0)

        xtb_next = load_xtb(0)
        sd_prev = None
        for tb in range(NTB):
            xtb = xtb_next
            if tb + 1 < NTB:
                xtb_next = load_xtb(tb * TB + TB)
            sd_cur = A_dt(tb, xtb)
            if sd_prev is not None:
                C_pre(sd_prev, 0)
                C_pre(sd_prev, 1)
            A_dt2(sd_cur)
            A_z(sd_cur)
            if sd_prev is not None:
                C_main(sd_prev, 0)
                C_main(sd_prev, 1)
            A_cmaj(sd_cur)
            A_conv(sd_cur)
            A_tr(sd_cur)
            sd_prev = sd_cur
        C_pre(sd_prev, 0)
        C_pre(sd_prev, 1)
        C_main(sd_prev, 0)
        C_main(sd_prev, 1)

        nc.sync.dma_start(d_OUT2, ssqall[:])

    nc.compile()
    return nc


# ================= host side =================

def _prep_core_inputs(x_b_T, in_w, conv_w, conv_b, dt_bias, A_log, Dp, norm_w,
                      out_w, proj_w_dir, hh):
    import ml_dtypes
    D_INNER = 1536
    zsel = slice(hh * HH, (hh + 1) * HH)
    xsel = slice(D_INNER + hh * HH, D_INNER + (hh + 1) * HH)
    Bsel = slice(2 * D_INNER, 2 * D_INNER + 16)
    Csel = slice(2 * D_INNER + 16, 2 * D_INNER + 32)
    dtsel = slice(2 * D_INNER + 32 + hh * NH, 2 * D_INNER + 32 + (hh + 1) * NH)

    # c-major rows: [x 768 | B 16 | C 16]
    Wc_rows = np.concatenate([in_w[xsel], in_w[Bsel], in_w[Csel]], 0)
    Wt_rows = in_w[zsel]
    Wdt_rows = in_w[dtsel]

    cwx = conv_w[hh * HH:(hh + 1) * HH]          # (768, 4) x-part
    cbx = conv_b[hh * HH:(hh + 1) * HH]
    cwB = conv_w[D_INNER:D_INNER + 16]
    cbB = conv_b[D_INNER:D_INNER + 16]
    cwC = conv_w[D_INNER + 16:D_INNER + 32]
    cbC = conv_b[D_INNER + 16:D_INNER + 32]

    DIAGW = np.zeros((D_CONV, NKT, P, P), np.float32)
    for k in range(D_CONV):
        for ct in range(NKT):
            DIAGW[k, ct][np.arange(P), np.arange(P)] = cwx[ct * P:(ct + 1) * P, k]
    DIAGB = np.zeros((D_CONV, P, D_STATE), np.float32)
    DIAGC = np.zeros((D_CONV, P, D_STATE), np.float32)
    for k in range(D_CONV):
        DIAGB[k][np.arange(16), np.arange(16)] = cwB[:, k]       # in-rows 0..15
        DIAGC[k][16 + np.arange(16), np.arange(16)] = cwC[:, k]  # in-rows 16..31
    CONVBX = np.zeros((P, NKT), np.float32)
    for ct in range(NKT):
        CONVBX[:, ct] = cbx[ct * P:(ct + 1) * P]

    a_neg = -np.exp(A_log[hh * NH:(hh + 1) * NH]).astype(np.float32)
    dtb = dt_bias[hh * NH:(hh + 1) * NH].astype(np.float32)
    TRIm = np.triu(np.ones((P, P), np.float32))
    RHSC = np.zeros((3 * NH, NH * TB), np.float32)
    for h in range(NH):
        for j in range(3):
            RHSC[h * 3 + j, h * TB:(h + 1) * TB] = -1.0
    DPBIG = np.repeat(Dp[hh * NH:(hh + 1) * NH].astype(np.float32), HEADDIM)[None, :] \
        .repeat(P, 0).copy()
    ow = (out_w * norm_w[None, :]).astype(np.float32)
    WCOMB = np.ascontiguousarray((proj_w_dir @ ow)[:, hh * HH:(hh + 1) * HH].T)

    bf = lambda a: np.ascontiguousarray(a).astype(ml_dtypes.bfloat16)
    f = np.ascontiguousarray
    return {
        "xT": f(x_b_T.astype(np.float32)),
        "Wc": f(Wc_rows.T.astype(np.float32)),
        "Wt": f(Wt_rows.T.astype(np.float32)),
        "Wdt": f(Wdt_rows.T.astype(np.float32)),
        "DIAGW": DIAGW, "DIAGB": DIAGB, "DIAGC": DIAGC,
        "CONVBX": CONVBX,
        "CONVBB": f(cbB.astype(np.float32)[:, None]),
        "CONVBC": f(cbC.astype(np.float32)[:, None]),
        "DTB_BC": f(np.repeat(dtb[None, :], P, 0)),
        "ANEG_BC": f(np.repeat(a_neg[None, :], P, 0)),
        "TRI": TRIm,
        "ONES3": bf(np.ones((3, TB), np.float32)),
        "RHSC": bf(RHSC),
        "DPBIG": bf(DPBIG),
        "WCOMB": bf(WCOMB),
    }


def make_in_maps(inputs):
    x = np.asarray(inputs["x"], np.float32)
    proj_w = np.asarray(inputs["proj_w"], np.float32)
    in_maps, core_meta = [], []
    for b in range(2):
        for d, pref in ((0, "f_"), (1, "b_")):
            xb = x[b] if d == 0 else x[b][::-1]
            for hh in range(2):
                g = lambda n: np.asarray(inputs[pref + n], np.float32)
                im = _prep_core_inputs(
                    np.ascontiguousarray(xb.T), g("in_w"), g("conv_w"), g("conv_b"),
                    g("dt_bias"), g("A_log"), g("Dp"), g("norm_w"), g("out_w"),
                    proj_w[:, d * D_MODEL:(d + 1) * D_MODEL], hh)
                in_maps.append(im)
                core_meta.append((b, d, hh))
    return in_maps, core_meta


def combine_outputs(results, core_meta, proj_b):
    out = np.zeros((2, SEQ, D_MODEL), np.float32)
    for b in range(2):
        for d in range(2):
            idx = [i for i, (bb, dd, _) in enumerate(core_meta) if bb == b and dd == d]
            part = sum(results[i]["OUT1"] for i in idx)
            ssq = sum(results[i]["OUT2"] for i in idx)       # (128, 16)
            ssq_t = ssq.T.reshape(SEQ)                        # t = ci*128 + p
            s = 1.0 / np.sqrt(ssq_t / 1536.0 + EPS)
            contrib = part * s[:, None]
            if d == 1:
                contrib = contrib[::-1]
            out[b] += contrib
    out += np.asarray(proj_b, np.float32)[None, None, :]
    return out


_NC_CACHE = {}


def kernel(**inputs):
    in_maps, core_meta = make_in_maps(inputs)
    if "nc" not in _NC_CACHE:
        _NC_CACHE["nc"] = build_program()
    nc = _NC_CACHE["nc"]
    res = run_bass_kernel_spmd(nc, in_maps, list(range(8)))
    return combine_outputs(res.results, core_meta, inputs["proj_b"])
